# revision 1
# baseline (speedup 1.0000x reference)
"""Trainium2 Bass kernel for a dense transformer block (QKV+gate proj, RoPE,
QK-RMSNorm, causal SDPA, output-RMSNorm + SiLU gate, output projection).

Sharding: tensor-parallel over heads across 8 NeuronCores (2 heads/core).
Wq/Wk/Wv/Wg split column-wise, attention fully local per core; the per-core
attention outputs are AllGathered and the output projection is computed with
Wo split column-wise (each core produces a 256-column slice of the output),
which replaces the row-parallel all-reduce with a much smaller all-gather.

All matmuls run as float32r (full fp32 storage, ~1 cycle/row on the PE).
"""

import os
import sys

for _p in ("/opt/trn_rl_repo", "/root/.axon_site/_ro/trn_rl_repo"):
    if os.path.isdir(_p) and _p not in sys.path:
        sys.path.insert(0, _p)

import numpy as np

import concourse.bass as bass
import concourse.mybir as mybir
from concourse import bacc
from concourse.bass_utils import run_bass_kernel_spmd
from concourse.tile import TileContext

B, T, HID = 2, 2048, 2048
H, D = 16, 128
NCORES = 8
HC = H // NCORES          # heads per core = 2
DC = HC * D               # 256 head-dims per core
BT = B * T                # 4096 tokens
KT = HID // 128           # 16 contraction tiles
EPS = 1e-5
SCALE = 1.0 / float(np.sqrt(D))
HALF_LN_D = 0.5 * float(np.log(D))  # ln(sqrt(128))
NEG = -3.0e38

F32 = mybir.dt.float32
F32R = mybir.dt.float32r
BF16 = mybir.dt.bfloat16
AF = mybir.ActivationFunctionType
ALU = mybir.AluOpType

# matmul dtype: bf16 (fast-weight-load, 2x DMA savings) by default;
# KERNEL_PRECISE=1 switches to float32r (near-fp32 accuracy, ~2x slower)
PRECISE = os.environ.get("KERNEL_PRECISE", "0") == "1"
MMDT = F32R if PRECISE else BF16
NPDT = None  # set in kernel(): np dtype matching MMDT

LAST_EXEC_TIME_NS = None
_CACHED_NC = None


def _proj_sweep(nc, tc, xT_r, w_aps, post):
    """One sweep over xT computing 2 matrices (4 head-groups) in transposed
    layout: psum[dhead 128, tok 512] double-buffered, accumulated over 16
    k-tiles."""
    with tc.tile_pool(name="sweep_w", bufs=1) as wpool, \
         tc.tile_pool(name="sweep_x", bufs=3) as xpool, \
         tc.tile_pool(name="sweep_ps", bufs=2, space="PSUM") as pps, \
         tc.tile_pool(name="sweep_t", bufs=2) as tpool:
        wsb = []
        for mi, w_ap in enumerate(w_aps):
            w_t = wpool.tile([128, KT, DC], MMDT, tag=f"w{mi}", name=f"w{mi}")
            nc.sync.dma_start(out=w_t, in_=w_ap)
            wsb.append(w_t)
        for nb in range(BT // 512):
            nbb, half = nb // 2, nb % 2
            ps = {}
            for mi in range(2):
                for m in range(HC):
                    ps[(mi, m)] = pps.tile([128, 512], F32,
                                           tag=f"pp{mi}{m}", name=f"pp{mi}{m}")
            for k in range(KT):
                xk = xpool.tile([128, 512], MMDT, tag=f"xk{half}", name="xk",
                                bufs=6)
                nc.sync.dma_start(
                    out=xk, in_=xT_r[k, nbb, :, half * 512:(half + 1) * 512])
                for mi in range(2):
                    for m in range(HC):
                        nc.tensor.matmul(
                            ps[(mi, m)], wsb[mi][:, k, m * 128:(m + 1) * 128],
                            xk, start=(k == 0), stop=(k == KT - 1))
            for mi in range(2):
                for m in range(HC):
                    post[mi](ps[(mi, m)], m, nb, tpool, pps, f"pp{mi}{m}")


def _final_body(nc, tc, wo_sb, yall, outT, ypool, opool, pps):
    for b in range(B):
        yall_b = yall[b].rearrange("(kt p) t -> p kt t", p=128)
        po = [pps.tile([128, 2048], F32, tag=f"po{m}", name=f"po{m}")
              for m in range(HC)]
        for kd in range(KT):
            ysl = ypool.tile([128, 2048], MMDT, tag="ysl", name="ysl",
                             bufs=4)
            nc.sync.dma_start(out=ysl, in_=yall_b[:, kd, :])
            for m in range(HC):
                lhsT = wo_sb[:, kd, m * 128:(m + 1) * 128]
                for tb in range(4):
                    nc.tensor.matmul(
                        po[m][:, tb * 512:(tb + 1) * 512], lhsT,
                        ysl[:, tb * 512:(tb + 1) * 512],
                        start=(kd == 0), stop=(kd == KT - 1))
        for m in range(HC):
            ot = opool.tile([128, 2048], F32, tag="ot", name="ot")
            nc.vector.tensor_copy(ot, po[m])
            nc.sync.dma_start(
                out=outT[m * 128:(m + 1) * 128, b * T:(b + 1) * T], in_=ot)


def _build_nc():
    nc = bacc.Bacc("TRN2", target_bir_lowering=False, debug=False,
                   num_devices=NCORES)

    xT = nc.dram_tensor("xT", [KT, BT // 1024, 128, 1024], MMDT,
                        kind="ExternalInput").ap()
    wq = nc.dram_tensor("wq", [128, KT, DC], MMDT, kind="ExternalInput").ap()
    wk = nc.dram_tensor("wk", [128, KT, DC], MMDT, kind="ExternalInput").ap()
    wv = nc.dram_tensor("wv", [128, KT, DC], MMDT, kind="ExternalInput").ap()
    wg = nc.dram_tensor("wg", [128, KT, DC], MMDT, kind="ExternalInput").ap()
    wo = nc.dram_tensor("wo", [128, KT, DC], MMDT, kind="ExternalInput").ap()
    cos2 = nc.dram_tensor("cos2", [128, BT], F32, kind="ExternalInput").ap()
    sin2 = nc.dram_tensor("sin2", [128, BT], F32, kind="ExternalInput").ap()
    negm = nc.dram_tensor("negm", [128, 128], F32, kind="ExternalInput").ap()
    ones_in = nc.dram_tensor("ones_in", [128, 128], MMDT, kind="ExternalInput").ap()
    ident_in = nc.dram_tensor("ident_in", [128, 128], F32, kind="ExternalInput").ap()
    qrw = nc.dram_tensor("qrw", [128, 1], F32, kind="ExternalInput").ap()
    krw = nc.dram_tensor("krw", [128, 1], F32, kind="ExternalInput").ap()

    outT = nc.dram_tensor("outT", [DC, BT], F32, kind="ExternalOutput").ap()
    ag_in = [nc.dram_tensor(f"ag_in{b}", [DC, T], MMDT).ap() for b in range(B)]
    gdram = nc.dram_tensor("gdram", [DC, BT], F32).ap()
    yall = [nc.dram_tensor(f"yall{b}", [NCORES * DC, T], MMDT,
                           addr_space="Shared").ap() for b in range(B)]

    xT_r = xT

    with TileContext(nc) as tc:
        with tc.tile_pool(name="const", bufs=1) as const:
            ones_r = const.tile([128, 128], MMDT)
            nc.sync.dma_start(out=ones_r, in_=ones_in)
            epsb = const.tile([128, 1], F32)
            nc.vector.memset(epsb, EPS)
            zerob = const.tile([128, 1], F32)
            nc.vector.memset(zerob, 0.0)

            with tc.tile_pool(name="persist", bufs=1) as persist:
                # final (rope+rms applied) qT/kT per head, f32r [d, b*t]
                qTf = [persist.tile([128, BT], MMDT, tag=f"qTf{m}",
                                    name=f"qTf{m}") for m in range(HC)]
                kTf = [persist.tile([128, BT], MMDT, tag=f"kTf{m}",
                                    name=f"kTf{m}") for m in range(HC)]

                # ---- sweep A: q, k (rope + rms fused into evacuation) ----
                with tc.tile_pool(name="ropec", bufs=1) as rp:
                    cos_sb = rp.tile([128, BT], F32)
                    nc.sync.dma_start(out=cos_sb, in_=cos2)
                    sin_sb = rp.tile([128, BT], F32)
                    nc.sync.dma_start(out=sin_sb, in_=sin2)
                    qrw_sb = rp.tile([128, 1], F32)
                    nc.sync.dma_start(out=qrw_sb, in_=qrw)
                    krw_sb = rp.tile([128, 1], F32)
                    nc.sync.dma_start(out=krw_sb, in_=krw)

                    def make_qk_post(dest, w_scalar):
                        def post(ps, m, nb, tpool, pps, pstag):
                            c0, c1 = nb * 512, (nb + 1) * 512
                            stage = tpool.tile([128, 512], F32, tag="stage",
                                               name="stage")
                            nc.scalar.copy(stage, ps)
                            sq = tpool.tile([128, 512], MMDT, tag="sq",
                                            name="sq")
                            nc.vector.tensor_mul(sq, stage, stage)
                            ss = pps.tile([128, 512], F32, tag=pstag, name="ss")
                            nc.tensor.matmul(ss, ones_r, sq,
                                             start=True, stop=True)
                            fac = tpool.tile([128, 512], F32, tag="fac",
                                             name="fac")
                            nc.scalar.activation(out=fac, in_=ss,
                                                 func=AF.Abs_reciprocal_sqrt,
                                                 scale=1.0 / float(D),
                                                 bias=epsb)
                            cc = cos_sb[:, c0:c1]
                            ssn = sin_sb[:, c0:c1]
                            sw = tpool.tile([128, 512], F32, tag="sw",
                                            name="sw")
                            nc.sync.dma_start(out=sw[0:64, :],
                                              in_=stage[64:128, :])
                            nc.sync.dma_start(out=sw[64:128, :],
                                              in_=stage[0:64, :])
                            u = tpool.tile([128, 512], F32, tag="t12",
                                           name="u")
                            w = tpool.tile([128, 512], F32, tag="t34",
                                           name="w")
                            nc.vector.tensor_mul(u, stage, cc)
                            nc.gpsimd.tensor_mul(w, sw, ssn)
                            ro = tpool.tile([128, 512], F32, tag="ro", name="ro")
                            nc.vector.tensor_sub(ro[0:64, :], u[0:64, :],
                                                 w[0:64, :])
                            nc.vector.tensor_add(ro[64:128, :], u[64:128, :],
                                                 w[64:128, :])
                            # dest = (ro * w[d]) * factor  (fused)
                            nc.vector.scalar_tensor_tensor(
                                out=dest[m][:, c0:c1], in0=ro, scalar=w_scalar,
                                in1=fac, op0=ALU.mult, op1=ALU.mult)
                        return post

                    _proj_sweep(nc, tc, xT_r, [wq, wk],
                                [make_qk_post(qTf, qrw_sb),
                                 make_qk_post(kTf, krw_sb)])

                # ---- sweep B: v (transpose to [tok, d]), gate (spill) ----
                v_sb = [persist.tile([128, BT // 128, 128], MMDT, tag=f"v{m}",
                                     name=f"v{m}") for m in range(HC)]
                with tc.tile_pool(name="identp", bufs=1) as ip:
                    ident = ip.tile([128, 128], F32)
                    nc.sync.dma_start(out=ident, in_=ident_in)

                    def v_post(ps, m, nb, tpool, pps, pstag):
                        stage = tpool.tile([128, 512], F32, tag="vstage",
                                           name="stage")
                        nc.scalar.copy(stage, ps)
                        for j in range(4):
                            tp = pps.tile([128, 128], F32, tag=pstag,
                                          name="tp")
                            nc.tensor.transpose(
                                tp, stage[:, j * 128:(j + 1) * 128], ident)
                            nc.vector.tensor_copy(v_sb[m][:, nb * 4 + j, :],
                                                  tp)

                    def g_post(ps, m, nb, tpool, pps, pstag):
                        gst = tpool.tile([128, 512], F32, tag="gst",
                                         name="gst")
                        nc.scalar.copy(gst, ps)
                        nc.sync.dma_start(
                            out=gdram[m * 128:(m + 1) * 128,
                                      nb * 512:(nb + 1) * 512],
                            in_=gst)

                    _proj_sweep(nc, tc, xT_r, [wv, wg], [v_post, g_post])

                # final-proj SBUF pools opened early (bf16 path) so the
                # b=0 projection can run while the b=1 AllGather is in flight
                if not PRECISE:
                    fin_ctx = [tc.tile_pool(name="fin_w", bufs=1),
                               tc.tile_pool(name="fin_y", bufs=3),
                               tc.tile_pool(name="fin_o", bufs=2)]
                    wpool, ypool, opool = [c.__enter__() for c in fin_ctx]
                    wo_sb = wpool.tile([128, KT, DC], MMDT, tag="wo")
                    nc.sync.dma_start(out=wo_sb, in_=wo)
                # ---- attention ----
                with tc.tile_pool(name="at_ps", bufs=1, space="PSUM") as pps, \
                     tc.tile_pool(name="at_ps2", bufs=2, space="PSUM") as pps2, \
                     tc.tile_pool(name="at_slab", bufs=1) as slab, \
                     tc.tile_pool(name="at_t", bufs=2) as tpool, \
                     tc.tile_pool(name="at_bh", bufs=1) as bhpool, \
                     tc.tile_pool(name="at_c", bufs=1) as acp:
                    negm_sb = acp.tile([128, 128], F32)
                    nc.sync.dma_start(out=negm_sb, in_=negm)
                    for b in range(B):
                      for m in range(HC):
                            t0 = b * T
                            ystash = bhpool.tile([128, T], F32, tag="bhC",
                                                 name="ystash")
                            wy = bhpool.tile([128, T], F32, tag="bhA",
                                             name="wy")
                            for qb in range(T // 512):
                                nk = 4 * (qb + 1)
                                es = []
                                for i2 in range(0, nk, 2):
                                    # S^T for two k-tiles -> one [128,1024] psum
                                    stp = pps2.tile([128, 1024], F32, tag="st",
                                                    name="stp")
                                    e = slab.tile([128, 1024], MMDT,
                                                  tag=f"es{i2 // 2}",
                                                  name=f"es{i2 // 2}")
                                    for j in range(2):
                                        i = i2 + j
                                        sl = slice(j * 512, (j + 1) * 512)
                                        nc.tensor.matmul(
                                            stp[:, sl],
                                            kTf[m][:, t0 + i * 128:
                                                   t0 + (i + 1) * 128],
                                            qTf[m][:, t0 + qb * 512:
                                                   t0 + (qb + 1) * 512],
                                            start=True, stop=True)
                                        q_off = i * 128 - qb * 512
                                        if q_off >= 0:
                                            nc.vector.tensor_add(
                                                stp[:, j * 512 + q_off:
                                                    j * 512 + q_off + 128],
                                                stp[:, j * 512 + q_off:
                                                    j * 512 + q_off + 128],
                                                negm_sb)
                                            if j == 0 and q_off > 0:
                                                # left of diagonal, first half:
                                                # excluded from exp; zero in e
                                                nc.gpsimd.memset(
                                                    e[:, 0:q_off], 0.0)
                                            if j == 1 and q_off > 0:
                                                # invalid region inside the exp
                                                # range: force exp() -> 0
                                                nc.vector.memset(
                                                    stp[:, 512:512 + q_off],
                                                    NEG)
                                    # exp over both halves at once
                                    q_off0 = i2 * 128 - qb * 512
                                    lo = max(0, q_off0)
                                    nc.scalar.activation(
                                        out=e[:, lo:], in_=stp[:, lo:],
                                        func=AF.Exp, scale=SCALE)
                                    es.append(e)
                                ytp = pps2.tile([128, 512], F32, tag="yt",
                                                name="ytp")
                                ssp = pps2.tile([128, 512], F32, tag="sexp",
                                                name="ssp", bufs=1)
                                for i2 in range(0, nk, 2):
                                    for j in range(2):
                                        i = i2 + j
                                        sl = slice(j * 512, (j + 1) * 512)
                                        nc.tensor.matmul(
                                            ytp, v_sb[m][:, b * 16 + i, :],
                                            es[i2 // 2][:, sl],
                                            start=(i == 0), stop=(i == nk - 1))
                                        nc.tensor.matmul(
                                            ssp, ones_r, es[i2 // 2][:, sl],
                                            start=(i == 0), stop=(i == nk - 1))
                                qsl = slice(qb * 512, (qb + 1) * 512)
                                sq = tpool.tile([128, 512], MMDT, tag="ysq",
                                                name="ysq")
                                nc.scalar.activation(out=sq, in_=ytp,
                                                     func=AF.Square)
                                nc.scalar.copy(ystash[:, qsl], ytp)
                                ssyp = pps.tile([128, 512], F32, tag="ssy",
                                                name="ssyp")
                                nc.tensor.matmul(ssyp, ones_r, sq,
                                                 start=True, stop=True)
                                s2t = tpool.tile([128, 512], F32, tag="s2",
                                                 name="s2t")
                                nc.scalar.activation(out=s2t, in_=ssp,
                                                     func=AF.Square)
                                # wy = ssy + (D*eps) * s^2
                                nc.vector.scalar_tensor_tensor(
                                    out=wy[:, qsl], in0=s2t,
                                    scalar=float(D) * EPS, in1=ssyp,
                                    op0=ALU.mult, op1=ALU.add)
                            # per-(b,h) tail: f = exp(-0.5 ln wy + ln(sqrt(D)))
                            fb2 = bhpool.tile([128, T], F32, tag="bhB",
                                              name="fb2")
                            nc.scalar.activation(out=fb2, in_=wy,
                                                 func=AF.Abs_reciprocal_sqrt,
                                                 scale=1.0 / float(D),
                                                 bias=zerob)
                            gld = bhpool.tile([128, T], F32, tag="bhF",
                                              name="gld")
                            nc.sync.dma_start(
                                out=gld,
                                in_=gdram[m * 128:(m + 1) * 128, t0:t0 + T])
                            sg = bhpool.tile([128, T], F32, tag="bhE",
                                             name="sg")
                            nc.scalar.activation(out=sg, in_=gld, func=AF.Silu)
                            yf1 = bhpool.tile([128, T], F32, tag="bhD",
                                              name="yf1")
                            nc.vector.tensor_mul(yf1, ystash, fb2)
                            yf = bhpool.tile([128, T], MMDT, tag="bhA",
                                             name="yf")
                            nc.vector.tensor_mul(yf, yf1, sg)
                            nc.sync.dma_start(
                                out=ag_in[b][m * 128:(m + 1) * 128, :],
                                in_=yf)
                      nc.gpsimd.collective_compute(
                          "AllGather", ALU.bypass,
                          ins=[ag_in[b]], outs=[yall[b]],
                          replica_groups=[list(range(NCORES))],
                      )


                if not PRECISE:
                    with tc.tile_pool(name="fin_ps", bufs=1,
                                      space="PSUM") as pps:
                        _final_body(nc, tc, wo_sb, yall, outT, ypool, opool,
                                    pps)
                    for c in reversed(fin_ctx):
                        c.__exit__(None, None, None)
            if PRECISE:
                with tc.tile_pool(name="fin_w", bufs=1) as wpool, \
                     tc.tile_pool(name="fin_y", bufs=3) as ypool, \
                     tc.tile_pool(name="fin_o", bufs=2) as opool, \
                     tc.tile_pool(name="fin_ps", bufs=1, space="PSUM") as pps:
                    wo_sb = wpool.tile([128, KT, DC], MMDT, tag="wo")
                    nc.sync.dma_start(out=wo_sb, in_=wo)
                    _final_body(nc, tc, wo_sb, yall, outT, ypool, opool, pps)
    nc.compile()
    return nc


def _get_nc():
    global _CACHED_NC
    if _CACHED_NC is None:
        _CACHED_NC = _build_nc()
    return _CACHED_NC


def kernel(x, Wq, Wk, Wv, Wg, Wo, q_rms_w, k_rms_w, o_norm_w):
    global LAST_EXEC_TIME_NS
    import ml_dtypes
    npdt = np.float32 if PRECISE else ml_dtypes.bfloat16
    x = np.asarray(x, dtype=np.float32)
    Wq = np.asarray(Wq, dtype=np.float32)
    Wk = np.asarray(Wk, dtype=np.float32)
    Wv = np.asarray(Wv, dtype=np.float32)
    Wg = np.asarray(Wg, dtype=np.float32)
    Wo = np.asarray(Wo, dtype=np.float32)
    q_rms_w = np.asarray(q_rms_w, dtype=np.float32)
    k_rms_w = np.asarray(k_rms_w, dtype=np.float32)
    o_norm_w = np.asarray(o_norm_w, dtype=np.float32)

    xT = x.reshape(BT, HID).T          # [HID, BT]
    # [KT, BT//1024, 128, 1024] contiguous chunks
    xt4 = np.ascontiguousarray(
        xT.reshape(KT, 128, BT // 1024, 1024).transpose(0, 2, 1, 3)).astype(npdt)
    # fold o_norm_w into Wo rows: (y*o_w) @ Wo == y @ (o_w[:,None]*Wo)
    wo_scaled = Wo * np.tile(o_norm_w, H)[:, None]

    inv = 1.0 / (10000.0 ** (np.arange(0, D, 2, dtype=np.float64) / D))
    pos = np.arange(T, dtype=np.float64)
    fr = pos[:, None] * inv[None, :]          # [T, 64]
    cosT = np.cos(fr).T.astype(np.float32)    # [64, T]
    sinT = np.sin(fr).T.astype(np.float32)
    cosbt = np.concatenate([cosT] * B, axis=1)
    sinbt = np.concatenate([sinT] * B, axis=1)
    cos2 = np.ascontiguousarray(np.vstack([cosbt, cosbt]))   # [128, BT]
    sin2 = np.ascontiguousarray(np.vstack([sinbt, sinbt]))

    kk, qq = np.meshgrid(np.arange(128), np.arange(128), indexing="ij")
    negm = np.where(kk <= qq, 0.0, NEG).astype(np.float32)
    ones128 = np.ones((128, 128), dtype=np.float32)
    ident = np.eye(128, dtype=np.float32)

    in_maps = []
    for c in range(NCORES):
        csl = slice(c * DC, (c + 1) * DC)
        def wt(wmat):
            # [HID, DC] -> [128, KT, DC] matching the SBUF tile layout
            return np.ascontiguousarray(
                wmat[:, csl].reshape(KT, 128, DC).transpose(1, 0, 2)).astype(npdt)
        in_maps.append({
            "xT": xt4,
            "wq": wt(Wq),
            "wk": wt(Wk),
            "wv": wt(Wv),
            "wg": wt(Wg),
            "wo": wt(wo_scaled),
            "cos2": cos2,
            "sin2": sin2,
            "negm": negm,
            "ones_in": ones128.astype(npdt),
            "ident_in": ident,
            "qrw": np.ascontiguousarray(q_rms_w.reshape(128, 1)),
            "krw": np.ascontiguousarray(k_rms_w.reshape(128, 1)),
        })

    nc = _get_nc()
    trace = os.environ.get("KERNEL_TRACE", "0") == "1"
    res = run_bass_kernel_spmd(nc, in_maps, list(range(NCORES)), trace=trace)
    LAST_EXEC_TIME_NS = res.exec_time_ns

    outT_full = np.concatenate([res.results[c]["outT"] for c in range(NCORES)],
                               axis=0)              # [2048 n, 4096 t]
    out = outT_full.T.reshape(B, T, HID)
    return np.ascontiguousarray(out)



# revision 3
# speedup vs baseline: 1.0508x; 1.0508x over previous
"""Trainium2 Bass kernel for a dense transformer block (QKV+gate proj, RoPE,
QK-RMSNorm, causal SDPA, output-RMSNorm + SiLU gate, output projection).

Sharding: tensor-parallel over heads across 8 NeuronCores (2 heads/core) for
projections+attention; token-parallel output projection with replicated Wo.
The per-core attention outputs are exchanged with 4 small AllToAlls (one per
batch-half, ~0.5 MB/rank wire each) instead of AllGathers, and each core
computes the full 2048 output columns for its own 4x128-token chunks.

Numerics: all matmuls bf16 (f32 accumulate). The output-RMSNorm eps term
(eps*denom^2) is dropped (rel contribution <1.2e-2 of the variance, validated
<3e-3 output error), which removes all softmax-denominator matmuls. rsqrt and
silu are computed via exp/ln so the whole kernel uses a single ACT table set
(natural_log_exp_and_others) -- no table-switch stalls.
"""

import os
import sys

for _p in ("/opt/trn_rl_repo", "/root/.axon_site/_ro/trn_rl_repo"):
    if os.path.isdir(_p) and _p not in sys.path:
        sys.path.insert(0, _p)

import numpy as np

import concourse.bass as bass
import concourse.mybir as mybir
from concourse import bacc
from concourse.bass_utils import run_bass_kernel_spmd
from concourse.tile import TileContext

B, T, HID = 2, 2048, 2048
H, D = 16, 128
NCORES = 8
HC = H // NCORES          # heads per core = 2
DC = HC * D               # 256 head-dims per core
BT = B * T                # 4096 tokens
KT = HID // 128           # 16 contraction tiles
SCALE = 1.0 / float(np.sqrt(D))
NEG = -3.0e38

F32 = mybir.dt.float32
BF16 = mybir.dt.bfloat16
AF = mybir.ActivationFunctionType
ALU = mybir.AluOpType

MMDT = BF16

LAST_EXEC_TIME_NS = None
_CACHED_NC = None


class _Bacc(bacc.Bacc):
    """Bacc with an ACT-table-set preference: serve Exp and Ln from the
    combined natural_log_exp_and_others set so alternating Ln/Exp chains
    (rsqrt-via-exp/ln, silu-via-exp/ln) don't thrash table loads. The
    greedy chooser otherwise picks exp_and_others for Exp and natural_log
    for Ln, inserting a ~2.7us ACT_TABLE_LOAD at every alternation."""

    def insert_act_table_loads(self):
        import bass_rust as _bass_rust
        from concourse.hw_specs import get_activation_tables
        has_activation = any(
            isinstance(i, mybir.InstActivation)
            for b in self.main_func.blocks
            for i in b.instructions
        )
        if not has_activation:
            return
        AFT = mybir.ActivationFunctionType
        tables = []
        for name, fns in get_activation_tables(self.m.arch).items():
            if name != "natural_log_exp_and_others":
                fns = fns - {AFT.Exp, AFT.Ln}
            tables.append((name, fns))
        _bass_rust.insert_act_table_loads(self, tables)


def _build_nc():
    nc = _Bacc("TRN2", target_bir_lowering=False, debug=False,
               num_devices=NCORES)

    xT = nc.dram_tensor("xT", [KT, BT // 1024, 128, 1024], MMDT,
                        kind="ExternalInput").ap()
    wq = nc.dram_tensor("wq", [128, KT, DC], MMDT, kind="ExternalInput").ap()
    wk = nc.dram_tensor("wk", [128, KT, DC], MMDT, kind="ExternalInput").ap()
    wv = nc.dram_tensor("wv", [128, KT, DC], MMDT, kind="ExternalInput").ap()
    wg = nc.dram_tensor("wg", [128, KT, DC], MMDT, kind="ExternalInput").ap()
    # full (replicated) Wo with o_norm folded: [128, kd-tile, 2048 cols]
    wo = nc.dram_tensor("wo", [128, KT, HID], MMDT, kind="ExternalInput").ap()
    cos2 = nc.dram_tensor("cos2", [128, T], F32, kind="ExternalInput").ap()
    sin2 = nc.dram_tensor("sin2", [128, T], F32, kind="ExternalInput").ap()
    negm = nc.dram_tensor("negm", [128, 128], F32, kind="ExternalInput").ap()
    ones_in = nc.dram_tensor("ones_in", [128, 128], MMDT,
                             kind="ExternalInput").ap()
    qrw = nc.dram_tensor("qrw", [128, 1], F32, kind="ExternalInput").ap()
    krw = nc.dram_tensor("krw", [128, 1], F32, kind="ExternalInput").ap()

    # output: per (b, half) a [128 tok, 2048] f32 chunk
    out_tok = nc.dram_tensor("out_tok", [B * 2, 128, HID], F32,
                             kind="ExternalOutput").ap()

    # AllToAll buffers: per (b, half): [8 shards(128 tok), 256 hd, 128 tok]
    a2a_in = [nc.dram_tensor(f"a2a_in{i}", [NCORES, DC, 128], MMDT).ap()
              for i in range(B * 2)]
    a2a_out = [nc.dram_tensor(f"a2a_out{i}", [NCORES, DC, 128], MMDT).ap()
               for i in range(B * 2)]

    with TileContext(nc) as tc:
        with tc.tile_pool(name="const", bufs=1) as const:
            ones_r = const.tile([128, 128], MMDT)
            nc.sync.dma_start(out=ones_r, in_=ones_in)
            onesb = const.tile([128, 1], F32)
            nc.vector.memset(onesb, 1.0)

            with tc.tile_pool(name="persist", bufs=1) as persist:
                # final (rope+rms applied) qT/kT per head [d, tok] bf16;
                # written raw by sweep A, rewritten in place by post-A
                qTf = [persist.tile([128, BT], MMDT, tag=f"qTf{m}",
                                    name=f"qTf{m}") for m in range(HC)]
                kTf = [persist.tile([128, BT], MMDT, tag=f"kTf{m}",
                                    name=f"kTf{m}") for m in range(HC)]
                # v in [tok, d] layout per head (filled by DMA transpose)
                v_sb = [persist.tile([128, BT // 128, 128], MMDT,
                                     tag=f"v{m}", name=f"v{m}")
                        for m in range(HC)]
                # silu(gate) per head [d, tok]
                sg_sb = [persist.tile([128, BT], MMDT, tag=f"sg{m}",
                                      name=f"sg{m}") for m in range(HC)]

                # ============ sweep A: q, k + fused post (rope+rms) ========
                with tc.tile_pool(name="swA_w", bufs=1) as wpool, \
                     tc.tile_pool(name="swA_x", bufs=8) as xpool, \
                     tc.tile_pool(name="swA_ps", bufs=1, space="PSUM") as pps, \
                     tc.tile_pool(name="swA_ss", bufs=2, space="PSUM") as pss, \
                     tc.tile_pool(name="swA_t", bufs=3) as tpool, \
                     tc.tile_pool(name="ropec", bufs=1) as rp:
                    wsb = []
                    for mi, w_ap in enumerate([wq, wk]):
                        w_t = wpool.tile([128, KT, DC], MMDT, tag=f"w{mi}",
                                         name=f"w{mi}")
                        nc.sync.dma_start(out=w_t, in_=w_ap)
                        wsb.append(w_t)
                    cos_sb = rp.tile([128, T], F32)
                    nc.sync.dma_start(out=cos_sb, in_=cos2)
                    sin_sb = rp.tile([128, T], F32)
                    nc.sync.dma_start(out=sin_sb, in_=sin2)
                    qrw_sb = rp.tile([128, 1], F32)
                    nc.sync.dma_start(out=qrw_sb, in_=qrw)
                    krw_sb = rp.tile([128, 1], F32)
                    nc.sync.dma_start(out=krw_sb, in_=krw)
                    wsc = [qrw_sb, krw_sb]

                    for nb in range(BT // 512):
                        nbb, half = nb // 2, nb % 2
                        c0 = nb * 512
                        ct0 = (nb * 512) % T       # within-batch token base
                        ps = {}
                        for mi in range(2):
                            for m in range(HC):
                                ps[(mi, m)] = pps.tile(
                                    [128, 512], F32, tag=f"pp{mi}{m}",
                                    name=f"pp{mi}{m}")
                        for k in range(KT):
                            xk = xpool.tile([128, 512], MMDT, tag="xk",
                                            name="xk")
                            nc.sync.dma_start(
                                out=xk,
                                in_=xT[k, nbb, :, half * 512:(half + 1) * 512])
                            for mi in range(2):
                                for m in range(HC):
                                    nc.tensor.matmul(
                                        ps[(mi, m)],
                                        wsb[mi][:, k, m * 128:(m + 1) * 128],
                                        xk, start=(k == 0), stop=(k == KT - 1))
                        # evacuate raw bf16 (same order as next block's MMs)
                        dests = [qTf, kTf]
                        for mi in range(2):
                            for m in range(HC):
                                nc.vector.tensor_copy(
                                    dests[mi][m][:, c0:c0 + 512], ps[(mi, m)])
                        # ---- post: rms factor + rope, in place ----
                        for mi in range(2):
                            for m in range(HC):
                                raw = dests[mi][m][:, c0:c0 + 512]
                                sq = tpool.tile([128, 512], MMDT, tag="sq",
                                                name="sq")
                                nc.vector.tensor_mul(sq, raw, raw)
                                ss = pss.tile([128, 512], F32, tag="ss",
                                              name="ss")
                                nc.tensor.matmul(ss, ones_r, sq,
                                                 start=True, stop=True)
                                # fac = rsqrt(ss/D) = exp(-0.5*ln(ss/D))
                                lnt = tpool.tile([128, 512], F32, tag="lnt",
                                                 name="lnt")
                                nc.scalar.activation(out=lnt, in_=ss,
                                                     func=AF.Ln,
                                                     scale=1.0 / float(D))
                                fac = tpool.tile([128, 512], F32, tag="fac",
                                                 name="fac")
                                nc.scalar.activation(out=fac, in_=lnt,
                                                     func=AF.Exp, scale=-0.5)
                                # rope: swap halves via SBUF DMA
                                sw = tpool.tile([128, 512], MMDT, tag="sw",
                                                name="sw")
                                nc.scalar.dma_start(out=sw[0:64, :],
                                                    in_=raw[64:128, :])
                                nc.scalar.dma_start(out=sw[64:128, :],
                                                    in_=raw[0:64, :])
                                cc = cos_sb[:, ct0:ct0 + 512]
                                ssn = sin_sb[:, ct0:ct0 + 512]
                                u = tpool.tile([128, 512], F32, tag="u",
                                               name="u")
                                w2 = tpool.tile([128, 512], F32, tag="w2",
                                                name="w2")
                                nc.vector.tensor_mul(u, raw, cc)
                                nc.gpsimd.tensor_mul(w2, sw, ssn)
                                ro = tpool.tile([128, 512], F32, tag="ro",
                                                name="ro")
                                nc.vector.tensor_sub(ro[0:64, :], u[0:64, :],
                                                     w2[0:64, :])
                                nc.vector.tensor_add(ro[64:128, :],
                                                     u[64:128, :],
                                                     w2[64:128, :])
                                # dest = (ro * w[d]) * fac  (fused, in place)
                                nc.vector.scalar_tensor_tensor(
                                    out=raw, in0=ro, scalar=wsc[mi],
                                    in1=fac, op0=ALU.mult, op1=ALU.mult)

                # ============ sweep B: gate, v ============
                with tc.tile_pool(name="swB_w", bufs=1) as wpool, \
                     tc.tile_pool(name="swB_x", bufs=8) as xpool, \
                     tc.tile_pool(name="swB_ps", bufs=2, space="PSUM") as pps, \
                     tc.tile_pool(name="swB_t", bufs=3) as tpool, \
                     tc.tile_pool(name="vflat", bufs=1) as vfp:
                    wsb = []
                    for mi, w_ap in enumerate([wg, wv]):
                        w_t = wpool.tile([128, KT, DC], MMDT, tag=f"w{mi}",
                                         name=f"w{mi}")
                        nc.sync.dma_start(out=w_t, in_=w_ap)
                        wsb.append(w_t)
                    vflat = [vfp.tile([128, BT], MMDT, tag=f"vf{m}",
                                      name=f"vf{m}") for m in range(HC)]

                    for nb in range(BT // 512):
                        nbb, half = nb // 2, nb % 2
                        c0 = nb * 512
                        ps = {}
                        for mi in range(2):
                            for m in range(HC):
                                ps[(mi, m)] = pps.tile(
                                    [128, 512], F32, tag=f"pp{mi}{m}",
                                    name=f"pp{mi}{m}")
                        for k in range(KT):
                            xk = xpool.tile([128, 512], MMDT, tag="xk",
                                            name="xk")
                            nc.sync.dma_start(
                                out=xk,
                                in_=xT[k, nbb, :, half * 512:(half + 1) * 512])
                            for mi in range(2):
                                for m in range(HC):
                                    nc.tensor.matmul(
                                        ps[(mi, m)],
                                        wsb[mi][:, k, m * 128:(m + 1) * 128],
                                        xk, start=(k == 0), stop=(k == KT - 1))
                        for m in range(HC):
                            # gate: silu via exp/ln (single ACT table set)
                            graw = tpool.tile([128, 512], F32, tag="graw",
                                              name="graw")
                            nc.vector.tensor_copy(graw, ps[(0, m)])
                            e1 = tpool.tile([128, 512], F32, tag="e1",
                                            name="e1")
                            nc.scalar.activation(out=e1, in_=graw,
                                                 func=AF.Exp, scale=-1.0)
                            l1 = tpool.tile([128, 512], F32, tag="l1",
                                            name="l1")
                            nc.scalar.activation(out=l1, in_=e1, func=AF.Ln,
                                                 scale=1.0, bias=onesb)
                            s1 = tpool.tile([128, 512], F32, tag="s1",
                                            name="s1")
                            nc.scalar.activation(out=s1, in_=l1, func=AF.Exp,
                                                 scale=-1.0)
                            nc.vector.tensor_mul(sg_sb[m][:, c0:c0 + 512],
                                                 graw, s1)
                            # v: evacuate flat, then DMA-transpose per 128
                            nc.vector.tensor_copy(vflat[m][:, c0:c0 + 512],
                                                  ps[(1, m)])
                        for m in range(HC):
                            for j in range(4):
                                jj = nb * 4 + j
                                nc.scalar.dma_start(
                                    out=v_sb[m][:, jj, :],
                                    in_=vflat[m][:, jj * 128:(jj + 1) * 128],
                                    transpose=True)

                # ============ attention + A2A + token-parallel out proj ====
                with tc.tile_pool(name="at_st", bufs=2, space="PSUM") as pst, \
                     tc.tile_pool(name="at_yt", bufs=2, space="PSUM") as pyt, \
                     tc.tile_pool(name="fin_ps", bufs=1, space="PSUM") as pfo, \
                     tc.tile_pool(name="at_es", bufs=4) as espool, \
                     tc.tile_pool(name="at_t", bufs=3) as tpool, \
                     tc.tile_pool(name="at_c", bufs=1) as acp, \
                     tc.tile_pool(name="fin_w", bufs=1) as fwp, \
                     tc.tile_pool(name="fin_y", bufs=2) as fyp, \
                     tc.tile_pool(name="fin_o", bufs=2) as fop:
                    negm_sb = acp.tile([128, 128], F32)
                    nc.sync.dma_start(out=negm_sb, in_=negm)
                    wo_sb = fwp.tile([128, KT, HID], MMDT, tag="wo")
                    nc.sync.dma_start(out=wo_sb, in_=wo)

                    for b in range(B):
                        t0 = b * T
                        for hf in range(2):
                            ai = b * 2 + hf
                            for qb in (2 * hf, 2 * hf + 1):
                                for m in range(HC):
                                    nk = 4 * (qb + 1)
                                    q0 = t0 + qb * 512
                                    ytp = pyt.tile([128, 512], F32, tag="yt",
                                                   name="ytp")
                                    for i2 in range(0, nk, 2):
                                        stp = pst.tile([128, 1024], F32,
                                                       tag="st", name="stp")
                                        e = espool.tile([128, 1024], MMDT,
                                                        tag="es", name="es")
                                        for j in range(2):
                                            i = i2 + j
                                            sl = slice(j * 512, (j + 1) * 512)
                                            nc.tensor.matmul(
                                                stp[:, sl],
                                                kTf[m][:, t0 + i * 128:
                                                       t0 + (i + 1) * 128],
                                                qTf[m][:, q0:q0 + 512],
                                                start=True, stop=True)
                                            q_off = i * 128 - qb * 512
                                            if q_off >= 0:
                                                nc.vector.tensor_add(
                                                    stp[:, j * 512 + q_off:
                                                        j * 512 + q_off + 128],
                                                    stp[:, j * 512 + q_off:
                                                        j * 512 + q_off + 128],
                                                    negm_sb)
                                                if j == 0 and q_off > 0:
                                                    nc.vector.memset(
                                                        e[:, 0:q_off], 0.0)
                                                if j == 1 and q_off > 0:
                                                    nc.vector.memset(
                                                        stp[:, 512:
                                                            512 + q_off], NEG)
                                        q_off0 = i2 * 128 - qb * 512
                                        lo = max(0, q_off0)
                                        nc.scalar.activation(
                                            out=e[:, lo:], in_=stp[:, lo:],
                                            func=AF.Exp, scale=SCALE)
                                        for j in range(2):
                                            i = i2 + j
                                            sl = slice(j * 512, (j + 1) * 512)
                                            nc.tensor.matmul(
                                                ytp, v_sb[m][:, b * 16 + i, :],
                                                e[:, sl], start=(i == 0),
                                                stop=(i == nk - 1))
                                    # tail: f = rsqrt(mean_d u^2), y=u*f*sg
                                    sq = tpool.tile([128, 512], MMDT,
                                                    tag="ysq", name="ysq")
                                    nc.scalar.activation(out=sq, in_=ytp,
                                                         func=AF.Square)
                                    ystash = tpool.tile([128, 512], MMDT,
                                                        tag="yst",
                                                        name="ystash")
                                    nc.vector.tensor_copy(ystash, ytp)
                                    ssy = pyt.tile([128, 512], F32, tag="yt",
                                                   name="ssy")
                                    nc.tensor.matmul(ssy, ones_r, sq,
                                                     start=True, stop=True)
                                    lny = tpool.tile([128, 512], F32,
                                                     tag="lny", name="lny")
                                    nc.scalar.activation(
                                        out=lny, in_=ssy, func=AF.Ln,
                                        scale=1.0 / float(D))
                                    fy = tpool.tile([128, 512], F32,
                                                    tag="fy", name="fy")
                                    nc.scalar.activation(out=fy, in_=lny,
                                                         func=AF.Exp,
                                                         scale=-0.5)
                                    yf1 = tpool.tile([128, 512], F32,
                                                     tag="yf1", name="yf1")
                                    nc.vector.tensor_mul(yf1, ystash, fy)
                                    yf = tpool.tile([128, 512], MMDT,
                                                    tag="yf", name="yf")
                                    nc.vector.tensor_mul(
                                        yf, yf1,
                                        sg_sb[m][:, t0 + qb * 512:
                                                 t0 + (qb + 1) * 512])
                                    # store into A2A input: 4 shards of 128
                                    sh0 = 4 * (qb % 2)
                                    nc.sync.dma_start(
                                        out=a2a_in[ai][sh0:sh0 + 4,
                                                       m * 128:(m + 1) * 128,
                                                       :].rearrange(
                                            "s p t -> p s t"),
                                        in_=yf)
                            nc.gpsimd.collective_compute(
                                "AllToAll", ALU.bypass,
                                ins=[a2a_in[ai]], outs=[a2a_out[ai]],
                                replica_groups=[list(range(NCORES))],
                            )
                            # token-parallel out projection for this chunk
                            yg = fyp.tile([128, KT, 128], MMDT, tag="yg",
                                          name="yg")
                            nc.sync.dma_start(
                                out=yg,
                                in_=a2a_out[ai].rearrange(
                                    "r (h p) t -> p (r h) t", p=128))
                            for cp in range(2):   # column passes of 1024
                                fo = [pfo.tile([128, 512], F32,
                                               tag=f"fo{cc}", name=f"fo{cc}")
                                      for cc in range(2)]
                                for kd in range(KT):
                                    for cc in range(2):
                                        col = cp * 1024 + cc * 512
                                        nc.tensor.matmul(
                                            fo[cc], yg[:, kd, :],
                                            wo_sb[:, kd, col:col + 512],
                                            start=(kd == 0),
                                            stop=(kd == KT - 1))
                                for cc in range(2):
                                    ot = fop.tile([128, 512], F32, tag="ot",
                                                  name="ot")
                                    nc.vector.tensor_copy(ot, fo[cc])
                                    nc.sync.dma_start(
                                        out=out_tok[ai, :,
                                                    cp * 1024 + cc * 512:
                                                    cp * 1024 + cc * 512
                                                    + 512],
                                        in_=ot)
    nc.compile()
    return nc


def _get_nc():
    global _CACHED_NC
    if _CACHED_NC is None:
        _CACHED_NC = _build_nc()
    return _CACHED_NC


def kernel(x, Wq, Wk, Wv, Wg, Wo, q_rms_w, k_rms_w, o_norm_w):
    global LAST_EXEC_TIME_NS
    import ml_dtypes
    npdt = ml_dtypes.bfloat16
    x = np.asarray(x, dtype=np.float32)
    Wq = np.asarray(Wq, dtype=np.float32)
    Wk = np.asarray(Wk, dtype=np.float32)
    Wv = np.asarray(Wv, dtype=np.float32)
    Wg = np.asarray(Wg, dtype=np.float32)
    Wo = np.asarray(Wo, dtype=np.float32)
    q_rms_w = np.asarray(q_rms_w, dtype=np.float32)
    k_rms_w = np.asarray(k_rms_w, dtype=np.float32)
    o_norm_w = np.asarray(o_norm_w, dtype=np.float32)

    xT = x.reshape(BT, HID).T          # [HID, BT]
    xt4 = np.ascontiguousarray(
        xT.reshape(KT, 128, BT // 1024, 1024).transpose(0, 2, 1, 3)).astype(npdt)
    # fold o_norm_w into Wo rows: (y*o_w) @ Wo == y @ (o_w[:,None]*Wo)
    wo_scaled = Wo * np.tile(o_norm_w, H)[:, None]
    wo_t = np.ascontiguousarray(
        wo_scaled.reshape(KT, 128, HID).transpose(1, 0, 2)).astype(npdt)

    inv = 1.0 / (10000.0 ** (np.arange(0, D, 2, dtype=np.float64) / D))
    pos = np.arange(T, dtype=np.float64)
    fr = pos[:, None] * inv[None, :]          # [T, 64]
    cosT = np.cos(fr).T.astype(np.float32)    # [64, T]
    sinT = np.sin(fr).T.astype(np.float32)
    cos2 = np.ascontiguousarray(np.vstack([cosT, cosT]))   # [128, T]
    sin2 = np.ascontiguousarray(np.vstack([sinT, sinT]))

    kk, qq = np.meshgrid(np.arange(128), np.arange(128), indexing="ij")
    negm = np.where(kk <= qq, 0.0, NEG).astype(np.float32)
    ones128 = np.ones((128, 128), dtype=np.float32)

    in_maps = []
    for c in range(NCORES):
        csl = slice(c * DC, (c + 1) * DC)

        def wt(wmat):
            # [HID, DC] -> [128, KT, DC] matching the SBUF tile layout
            return np.ascontiguousarray(
                wmat[:, csl].reshape(KT, 128, DC).transpose(1, 0, 2)).astype(npdt)
        in_maps.append({
            "xT": xt4,
            "wq": wt(Wq),
            "wk": wt(Wk),
            "wv": wt(Wv),
            "wg": wt(Wg),
            "wo": wo_t,
            "cos2": cos2,
            "sin2": sin2,
            "negm": negm,
            "ones_in": ones128.astype(npdt),
            "qrw": np.ascontiguousarray(q_rms_w.reshape(128, 1)),
            "krw": np.ascontiguousarray(k_rms_w.reshape(128, 1)),
        })

    nc = _get_nc()
    trace = os.environ.get("KERNEL_TRACE", "0") == "1"
    res = run_bass_kernel_spmd(nc, in_maps, list(range(NCORES)), trace=trace)
    LAST_EXEC_TIME_NS = res.exec_time_ns

    out = np.empty((B, T, HID), dtype=np.float32)
    for c in range(NCORES):
        ot = res.results[c]["out_tok"]        # [4, 128, 2048]
        for b in range(B):
            for hf in range(2):
                t0 = hf * 1024 + c * 128
                out[b, t0:t0 + 128, :] = ot[b * 2 + hf]
    return out


# revision 4
# speedup vs baseline: 1.2956x; 1.2330x over previous
"""Trainium2 Bass kernel for a dense transformer block (QKV+gate proj, RoPE,
QK-RMSNorm, causal SDPA, output-RMSNorm + SiLU gate, output projection).

Sharding: tensor-parallel over heads across 8 NeuronCores (2 heads/core) for
projections+attention; token-parallel output projection with replicated Wo.
The per-core attention outputs are exchanged with 4 small AllToAlls (one per
batch-half, ~0.5 MB/rank wire each) instead of AllGathers, and each core
computes the full 2048 output columns for its own 4x128-token chunks. Each
final projection is traced one attention-quarter after its AllToAll so the
strict PE FIFO never stalls on a collective.

Numerics: all matmuls bf16 (f32 accumulate). The output-RMSNorm eps term
(eps*denom^2) is dropped (validated <3e-3 output error), which removes all
softmax-denominator matmuls. rsqrt and silu are computed via exp/ln so the
whole kernel uses a single ACT table set (natural_log_exp_and_others).
"""

import os
import sys

for _p in ("/opt/trn_rl_repo", "/root/.axon_site/_ro/trn_rl_repo"):
    if os.path.isdir(_p) and _p not in sys.path:
        sys.path.insert(0, _p)

import numpy as np

import concourse.bass as bass
import concourse.mybir as mybir
from concourse import bacc
from concourse.bass_utils import run_bass_kernel_spmd
from concourse.tile import TileContext

B, T, HID = 2, 2048, 2048
H, D = 16, 128
NCORES = 8
HC = H // NCORES          # heads per core = 2
DC = HC * D               # 256 head-dims per core
BT = B * T                # 4096 tokens
KT = HID // 128           # 16 contraction tiles
SCALE = 1.0 / float(np.sqrt(D))
NEG = -3.0e38

F32 = mybir.dt.float32
BF16 = mybir.dt.bfloat16
AF = mybir.ActivationFunctionType
ALU = mybir.AluOpType

MMDT = BF16

LAST_EXEC_TIME_NS = None
_CACHED_NC = None


class _Bacc(bacc.Bacc):
    """Bacc with an ACT-table-set preference: serve Exp and Ln from the
    combined natural_log_exp_and_others set so alternating Ln/Exp chains
    (rsqrt-via-exp/ln, silu-via-exp/ln) don't thrash table loads."""

    def insert_act_table_loads(self):
        import bass_rust as _bass_rust
        from concourse.hw_specs import get_activation_tables
        has_activation = any(
            isinstance(i, mybir.InstActivation)
            for b in self.main_func.blocks
            for i in b.instructions
        )
        if not has_activation:
            return
        AFT = mybir.ActivationFunctionType
        tables = []
        for name, fns in get_activation_tables(self.m.arch).items():
            if name != "natural_log_exp_and_others":
                fns = fns - {AFT.Exp, AFT.Ln}
            tables.append((name, fns))
        _bass_rust.insert_act_table_loads(self, tables)


def _build_nc():
    nc = _Bacc("TRN2", target_bir_lowering=False, debug=False,
               num_devices=NCORES)

    xT = nc.dram_tensor("xT", [KT, BT // 1024, 128, 1024], MMDT,
                        kind="ExternalInput").ap()
    wq = nc.dram_tensor("wq", [128, KT, DC], MMDT, kind="ExternalInput").ap()
    wk = nc.dram_tensor("wk", [128, KT, DC], MMDT, kind="ExternalInput").ap()
    wv = nc.dram_tensor("wv", [128, KT, DC], MMDT, kind="ExternalInput").ap()
    wg = nc.dram_tensor("wg", [128, KT, DC], MMDT, kind="ExternalInput").ap()
    # full (replicated) Wo with o_norm folded: [128, kd-tile, 2048 cols]
    wo = nc.dram_tensor("wo", [128, KT, HID], MMDT, kind="ExternalInput").ap()
    cos2 = nc.dram_tensor("cos2", [128, T], F32, kind="ExternalInput").ap()
    sin2 = nc.dram_tensor("sin2", [128, T], F32, kind="ExternalInput").ap()
    negm = nc.dram_tensor("negm", [128, 128], F32, kind="ExternalInput").ap()
    ones_in = nc.dram_tensor("ones_in", [128, 128], MMDT,
                             kind="ExternalInput").ap()
    qrw = nc.dram_tensor("qrw", [128, 1], F32, kind="ExternalInput").ap()
    krw = nc.dram_tensor("krw", [128, 1], F32, kind="ExternalInput").ap()

    # output: per (b, half) a [128 tok, 2048] f32 chunk
    out_tok = nc.dram_tensor("out_tok", [B * 2, 128, HID], F32,
                             kind="ExternalOutput").ap()

    # AllToAll buffers: per (b, half): [8 shards(128 tok), 256 hd, 128 tok]
    a2a_in = [nc.dram_tensor(f"a2a_in{i}", [NCORES, DC, 128], MMDT).ap()
              for i in range(B * 2)]
    a2a_out = [nc.dram_tensor(f"a2a_out{i}", [NCORES, DC, 128], MMDT).ap()
               for i in range(B * 2)]

    with TileContext(nc) as tc:
        with tc.tile_pool(name="const", bufs=1) as const:
            ones_r = const.tile([128, 128], MMDT)
            nc.sync.dma_start(out=ones_r, in_=ones_in)
            onesb = const.tile([128, 1], F32)
            nc.vector.memset(onesb, 1.0)

            with tc.tile_pool(name="persist", bufs=1) as persist, \
                 tc.tile_pool(name="weights", bufs=1) as wpool:
                # all projection weights loaded once, used by both sweeps
                wsb = {}
                for wn, w_ap in (("q", wq), ("k", wk), ("g", wg), ("v", wv)):
                    w_t = wpool.tile([128, KT, DC], MMDT, tag=f"w{wn}",
                                     name=f"w{wn}")
                    nc.sync.dma_start(out=w_t, in_=w_ap)
                    wsb[wn] = w_t

                # final (rope+rms applied) qT/kT per head [d, tok] bf16;
                # written raw by sweep A, rewritten in place by post-A
                qTf = [persist.tile([128, BT], MMDT, tag=f"qTf{m}",
                                    name=f"qTf{m}") for m in range(HC)]
                kTf = [persist.tile([128, BT], MMDT, tag=f"kTf{m}",
                                    name=f"kTf{m}") for m in range(HC)]
                # v in [tok, d] layout per head (filled by DMA transpose)
                v_sb = [persist.tile([128, BT // 128, 128], MMDT,
                                     tag=f"v{m}", name=f"v{m}")
                        for m in range(HC)]
                # silu(gate) per head [d, tok]
                sg_sb = [persist.tile([128, BT], MMDT, tag=f"sg{m}",
                                      name=f"sg{m}") for m in range(HC)]

                # ============ sweep A: q, k + fused post (rope+rms) ========
                with tc.tile_pool(name="swA_x", bufs=5) as xpool, \
                     tc.tile_pool(name="swA_ps", bufs=1, space="PSUM") as pps, \
                     tc.tile_pool(name="swA_ss", bufs=2, space="PSUM") as pss, \
                     tc.tile_pool(name="swA_t", bufs=3) as tpool, \
                     tc.tile_pool(name="ropec", bufs=1) as rp:
                    cos_sb = rp.tile([128, T], F32)
                    nc.sync.dma_start(out=cos_sb, in_=cos2)
                    sin_sb = rp.tile([128, T], F32)
                    nc.sync.dma_start(out=sin_sb, in_=sin2)
                    qrw_sb = rp.tile([128, 1], F32)
                    nc.sync.dma_start(out=qrw_sb, in_=qrw)
                    krw_sb = rp.tile([128, 1], F32)
                    nc.sync.dma_start(out=krw_sb, in_=krw)
                    wsc = [qrw_sb, krw_sb]

                    for nbb in range(BT // 1024):
                        xch = []
                        for c in range(4):
                            xc = xpool.tile([128, 4, 1024], MMDT, tag="xc",
                                            name="xc")
                            nc.sync.dma_start(
                                out=xc,
                                in_=xT[4 * c:4 * c + 4, nbb, :, :].rearrange(
                                    "k p t -> p k t"))
                            xch.append(xc)
                        for half in range(2):
                            nb = 2 * nbb + half
                            c0 = nb * 512
                            ct0 = c0 % T       # within-batch token base
                            hs = slice(half * 512, (half + 1) * 512)
                            ps = {}
                            for mi in range(2):
                                for m in range(HC):
                                    ps[(mi, m)] = pps.tile(
                                        [128, 512], F32, tag=f"pp{mi}{m}",
                                        name=f"pp{mi}{m}")
                            for k in range(KT):
                                xk = xch[k // 4][:, k % 4, hs]
                                for mi, wn in enumerate(("q", "k")):
                                    for m in range(HC):
                                        nc.tensor.matmul(
                                            ps[(mi, m)],
                                            wsb[wn][:, k,
                                                    m * 128:(m + 1) * 128],
                                            xk, start=(k == 0),
                                            stop=(k == KT - 1))
                            # evacuate raw bf16 (same order as next block MMs)
                            dests = [qTf, kTf]
                            for mi in range(2):
                                for m in range(HC):
                                    nc.vector.tensor_copy(
                                        dests[mi][m][:, c0:c0 + 512],
                                        ps[(mi, m)])
                            # ---- post: rms factor + rope, in place ----
                            for mi in range(2):
                                for m in range(HC):
                                    raw = dests[mi][m][:, c0:c0 + 512]
                                    sq = tpool.tile([128, 512], MMDT,
                                                    tag="sq", name="sq")
                                    nc.vector.tensor_mul(sq, raw, raw)
                                    ss = pss.tile([128, 512], F32, tag="ss",
                                                  name="ss")
                                    nc.tensor.matmul(ss, ones_r, sq,
                                                     start=True, stop=True)
                                    # fac = rsqrt(ss/D) = exp(-0.5*ln(ss/D))
                                    lnt = tpool.tile([128, 512], F32,
                                                     tag="lnt", name="lnt")
                                    nc.scalar.activation(
                                        out=lnt, in_=ss, func=AF.Ln,
                                        scale=1.0 / float(D))
                                    fac = tpool.tile([128, 512], F32,
                                                     tag="fac", name="fac")
                                    nc.scalar.activation(
                                        out=fac, in_=lnt, func=AF.Exp,
                                        scale=-0.5)
                                    # rope: swap halves via SBUF DMA
                                    sw = tpool.tile([128, 512], MMDT,
                                                    tag="sw", name="sw")
                                    nc.scalar.dma_start(out=sw[0:64, :],
                                                        in_=raw[64:128, :])
                                    nc.scalar.dma_start(out=sw[64:128, :],
                                                        in_=raw[0:64, :])
                                    cc = cos_sb[:, ct0:ct0 + 512]
                                    ssn = sin_sb[:, ct0:ct0 + 512]
                                    u = tpool.tile([128, 512], F32, tag="u",
                                                   name="u")
                                    w2 = tpool.tile([128, 512], F32,
                                                    tag="w2", name="w2")
                                    nc.vector.tensor_mul(u, raw, cc)
                                    nc.gpsimd.tensor_mul(w2, sw, ssn)
                                    ro = tpool.tile([128, 512], F32,
                                                    tag="ro", name="ro")
                                    nc.vector.tensor_sub(ro[0:64, :],
                                                         u[0:64, :],
                                                         w2[0:64, :])
                                    nc.vector.tensor_add(ro[64:128, :],
                                                         u[64:128, :],
                                                         w2[64:128, :])
                                    # dest = (ro * w[d]) * fac (fused)
                                    nc.vector.scalar_tensor_tensor(
                                        out=raw, in0=ro, scalar=wsc[mi],
                                        in1=fac, op0=ALU.mult, op1=ALU.mult)

                # ============ sweep B: gate, v ============
                with tc.tile_pool(name="swB_x", bufs=5) as xpool, \
                     tc.tile_pool(name="swB_ps", bufs=2, space="PSUM") as pps, \
                     tc.tile_pool(name="swB_t", bufs=3) as tpool, \
                     tc.tile_pool(name="vflat", bufs=1) as vfp:
                    vflat = [vfp.tile([128, BT], MMDT, tag=f"vf{m}",
                                      name=f"vf{m}") for m in range(HC)]

                    for nbb in range(BT // 1024):
                        xch = []
                        for c in range(4):
                            xc = xpool.tile([128, 4, 1024], MMDT, tag="xc",
                                            name="xc")
                            nc.sync.dma_start(
                                out=xc,
                                in_=xT[4 * c:4 * c + 4, nbb, :, :].rearrange(
                                    "k p t -> p k t"))
                            xch.append(xc)
                        for half in range(2):
                            nb = 2 * nbb + half
                            c0 = nb * 512
                            hs = slice(half * 512, (half + 1) * 512)
                            ps = {}
                            for mi in range(2):
                                for m in range(HC):
                                    ps[(mi, m)] = pps.tile(
                                        [128, 512], F32, tag=f"pp{mi}{m}",
                                        name=f"pp{mi}{m}")
                            for k in range(KT):
                                xk = xch[k // 4][:, k % 4, hs]
                                for mi, wn in enumerate(("g", "v")):
                                    for m in range(HC):
                                        nc.tensor.matmul(
                                            ps[(mi, m)],
                                            wsb[wn][:, k,
                                                    m * 128:(m + 1) * 128],
                                            xk, start=(k == 0),
                                            stop=(k == KT - 1))
                            for m in range(HC):
                                # gate: silu via exp/ln (single table set)
                                graw = tpool.tile([128, 512], F32,
                                                  tag="graw", name="graw")
                                nc.vector.tensor_copy(graw, ps[(0, m)])
                                e1 = tpool.tile([128, 512], F32, tag="e1",
                                                name="e1")
                                nc.scalar.activation(out=e1, in_=graw,
                                                     func=AF.Exp, scale=-1.0)
                                l1 = tpool.tile([128, 512], F32, tag="l1",
                                                name="l1")
                                nc.scalar.activation(out=l1, in_=e1,
                                                     func=AF.Ln,
                                                     scale=1.0, bias=onesb)
                                s1 = tpool.tile([128, 512], F32, tag="s1",
                                                name="s1")
                                nc.scalar.activation(out=s1, in_=l1,
                                                     func=AF.Exp, scale=-1.0)
                                nc.vector.tensor_mul(
                                    sg_sb[m][:, c0:c0 + 512], graw, s1)
                                # v: evacuate flat, DMA-transpose per 128
                                nc.vector.tensor_copy(
                                    vflat[m][:, c0:c0 + 512], ps[(1, m)])
                            for m in range(HC):
                                for j in range(4):
                                    jj = nb * 4 + j
                                    nc.sync.dma_start(
                                        out=v_sb[m][:, jj, :],
                                        in_=vflat[m][:,
                                                     jj * 128:(jj + 1) * 128],
                                        transpose=True)

                # ============ attention + A2A + token-parallel out proj ====
                with tc.tile_pool(name="at_st", bufs=2, space="PSUM") as pst, \
                     tc.tile_pool(name="at_yt", bufs=2, space="PSUM") as pyt, \
                     tc.tile_pool(name="fin_ps", bufs=1, space="PSUM") as pfo, \
                     tc.tile_pool(name="at_es", bufs=4) as espool, \
                     tc.tile_pool(name="at_t", bufs=3) as tpool, \
                     tc.tile_pool(name="at_c", bufs=1) as acp, \
                     tc.tile_pool(name="fin_w", bufs=1) as fwp, \
                     tc.tile_pool(name="fin_y", bufs=2) as fyp, \
                     tc.tile_pool(name="fin_o", bufs=2) as fop:
                    negm_sb = acp.tile([128, 128], F32)
                    nc.sync.dma_start(out=negm_sb, in_=negm)
                    wo_sb = fwp.tile([128, KT, HID], MMDT, tag="wo")
                    nc.sync.dma_start(out=wo_sb, in_=wo)

                    def attention_quarter(b, hf):
                        t0 = b * T
                        ai = b * 2 + hf
                        for qb in (2 * hf, 2 * hf + 1):
                            for m in range(HC):
                                nk = 4 * (qb + 1)
                                q0 = t0 + qb * 512
                                ytp = pyt.tile([128, 512], F32, tag="yt",
                                               name="ytp")
                                for i2 in range(0, nk, 2):
                                    stp = pst.tile([128, 1024], F32,
                                                   tag="st", name="stp")
                                    e = espool.tile([128, 1024], MMDT,
                                                    tag="es", name="es")
                                    for j in range(2):
                                        i = i2 + j
                                        sl = slice(j * 512, (j + 1) * 512)
                                        nc.tensor.matmul(
                                            stp[:, sl],
                                            kTf[m][:, t0 + i * 128:
                                                   t0 + (i + 1) * 128],
                                            qTf[m][:, q0:q0 + 512],
                                            start=True, stop=True)
                                        q_off = i * 128 - qb * 512
                                        if q_off >= 0:
                                            nc.vector.tensor_add(
                                                stp[:, j * 512 + q_off:
                                                    j * 512 + q_off + 128],
                                                stp[:, j * 512 + q_off:
                                                    j * 512 + q_off + 128],
                                                negm_sb)
                                            if j == 0 and q_off > 0:
                                                nc.vector.memset(
                                                    e[:, 0:q_off], 0.0)
                                            if j == 1 and q_off > 0:
                                                nc.vector.memset(
                                                    stp[:, 512:512 + q_off],
                                                    NEG)
                                    q_off0 = i2 * 128 - qb * 512
                                    lo = max(0, q_off0)
                                    nc.scalar.activation(
                                        out=e[:, lo:], in_=stp[:, lo:],
                                        func=AF.Exp, scale=SCALE)
                                    for j in range(2):
                                        i = i2 + j
                                        sl = slice(j * 512, (j + 1) * 512)
                                        nc.tensor.matmul(
                                            ytp, v_sb[m][:, b * 16 + i, :],
                                            e[:, sl], start=(i == 0),
                                            stop=(i == nk - 1))
                                # tail: f = rsqrt(mean_d u^2), y = u*f*sg
                                sq = tpool.tile([128, 512], MMDT,
                                                tag="ysq", name="ysq")
                                nc.scalar.activation(out=sq, in_=ytp,
                                                     func=AF.Square)
                                ystash = tpool.tile([128, 512], MMDT,
                                                    tag="yst", name="ystash")
                                nc.vector.tensor_copy(ystash, ytp)
                                ssy = pyt.tile([128, 512], F32, tag="yt",
                                               name="ssy")
                                nc.tensor.matmul(ssy, ones_r, sq,
                                                 start=True, stop=True)
                                lny = tpool.tile([128, 512], F32,
                                                 tag="lny", name="lny")
                                nc.scalar.activation(out=lny, in_=ssy,
                                                     func=AF.Ln,
                                                     scale=1.0 / float(D))
                                fy = tpool.tile([128, 512], F32,
                                                tag="fy", name="fy")
                                nc.scalar.activation(out=fy, in_=lny,
                                                     func=AF.Exp, scale=-0.5)
                                yf1 = tpool.tile([128, 512], F32,
                                                 tag="yf1", name="yf1")
                                nc.vector.tensor_mul(yf1, ystash, fy)
                                yf = tpool.tile([128, 512], MMDT,
                                                tag="yf", name="yf")
                                nc.vector.tensor_mul(
                                    yf, yf1,
                                    sg_sb[m][:, t0 + qb * 512:
                                             t0 + (qb + 1) * 512])
                                # store into A2A input: 4 shards of 128
                                sh0 = 4 * (qb % 2)
                                nc.sync.dma_start(
                                    out=a2a_in[ai][sh0:sh0 + 4,
                                                   m * 128:(m + 1) * 128,
                                                   :].rearrange(
                                        "s p t -> p s t"),
                                    in_=yf)
                        nc.gpsimd.collective_compute(
                            "AllToAll", ALU.bypass,
                            ins=[a2a_in[ai]], outs=[a2a_out[ai]],
                            replica_groups=[list(range(NCORES))],
                        )

                    def final_chunk(ai):
                        # token-parallel out projection for chunk ai
                        yg = fyp.tile([128, KT, 128], MMDT, tag="yg",
                                      name="yg")
                        nc.sync.dma_start(
                            out=yg,
                            in_=a2a_out[ai].rearrange(
                                "r (h p) t -> p (r h) t", p=128))
                        for cp in range(2):   # column passes of 1024
                            fo = [pfo.tile([128, 512], F32,
                                           tag=f"fo{cc}", name=f"fo{cc}")
                                  for cc in range(2)]
                            for kd in range(KT):
                                for cc in range(2):
                                    col = cp * 1024 + cc * 512
                                    nc.tensor.matmul(
                                        fo[cc], yg[:, kd, :],
                                        wo_sb[:, kd, col:col + 512],
                                        start=(kd == 0),
                                        stop=(kd == KT - 1))
                            for cc in range(2):
                                ot = fop.tile([128, 512], F32, tag="ot",
                                              name="ot")
                                nc.vector.tensor_copy(ot, fo[cc])
                                nc.sync.dma_start(
                                    out=out_tok[ai, :,
                                                cp * 1024 + cc * 512:
                                                cp * 1024 + cc * 512 + 512],
                                    in_=ot)

                    # finals delayed one quarter behind their A2A so the
                    # strict PE FIFO never waits on an in-flight collective
                    quarters = [(b, hf) for b in range(B) for hf in range(2)]
                    for qi, (b, hf) in enumerate(quarters):
                        attention_quarter(b, hf)
                        if qi >= 1:
                            final_chunk(qi - 1)
                    final_chunk(3)
    nc.compile()
    return nc


def _get_nc():
    global _CACHED_NC
    if _CACHED_NC is None:
        _CACHED_NC = _build_nc()
    return _CACHED_NC


def kernel(x, Wq, Wk, Wv, Wg, Wo, q_rms_w, k_rms_w, o_norm_w):
    global LAST_EXEC_TIME_NS
    import ml_dtypes
    npdt = ml_dtypes.bfloat16
    x = np.asarray(x, dtype=np.float32)
    Wq = np.asarray(Wq, dtype=np.float32)
    Wk = np.asarray(Wk, dtype=np.float32)
    Wv = np.asarray(Wv, dtype=np.float32)
    Wg = np.asarray(Wg, dtype=np.float32)
    Wo = np.asarray(Wo, dtype=np.float32)
    q_rms_w = np.asarray(q_rms_w, dtype=np.float32)
    k_rms_w = np.asarray(k_rms_w, dtype=np.float32)
    o_norm_w = np.asarray(o_norm_w, dtype=np.float32)

    xT = x.reshape(BT, HID).T          # [HID, BT]
    xt4 = np.ascontiguousarray(
        xT.reshape(KT, 128, BT // 1024, 1024).transpose(0, 2, 1, 3)).astype(npdt)
    # fold o_norm_w into Wo rows: (y*o_w) @ Wo == y @ (o_w[:,None]*Wo)
    wo_scaled = Wo * np.tile(o_norm_w, H)[:, None]
    wo_t = np.ascontiguousarray(
        wo_scaled.reshape(KT, 128, HID).transpose(1, 0, 2)).astype(npdt)

    inv = 1.0 / (10000.0 ** (np.arange(0, D, 2, dtype=np.float64) / D))
    pos = np.arange(T, dtype=np.float64)
    fr = pos[:, None] * inv[None, :]          # [T, 64]
    cosT = np.cos(fr).T.astype(np.float32)    # [64, T]
    sinT = np.sin(fr).T.astype(np.float32)
    cos2 = np.ascontiguousarray(np.vstack([cosT, cosT]))   # [128, T]
    sin2 = np.ascontiguousarray(np.vstack([sinT, sinT]))

    kk, qq = np.meshgrid(np.arange(128), np.arange(128), indexing="ij")
    negm = np.where(kk <= qq, 0.0, NEG).astype(np.float32)
    ones128 = np.ones((128, 128), dtype=np.float32)

    in_maps = []
    for c in range(NCORES):
        csl = slice(c * DC, (c + 1) * DC)

        def wt(wmat):
            # [HID, DC] -> [128, KT, DC] matching the SBUF tile layout
            return np.ascontiguousarray(
                wmat[:, csl].reshape(KT, 128, DC).transpose(1, 0, 2)).astype(npdt)
        in_maps.append({
            "xT": xt4,
            "wq": wt(Wq),
            "wk": wt(Wk),
            "wv": wt(Wv),
            "wg": wt(Wg),
            "wo": wo_t,
            "cos2": cos2,
            "sin2": sin2,
            "negm": negm,
            "ones_in": ones128.astype(npdt),
            "qrw": np.ascontiguousarray(q_rms_w.reshape(128, 1)),
            "krw": np.ascontiguousarray(k_rms_w.reshape(128, 1)),
        })

    nc = _get_nc()
    trace = os.environ.get("KERNEL_TRACE", "0") == "1"
    res = run_bass_kernel_spmd(nc, in_maps, list(range(NCORES)), trace=trace)
    LAST_EXEC_TIME_NS = res.exec_time_ns

    out = np.empty((B, T, HID), dtype=np.float32)
    for c in range(NCORES):
        ot = res.results[c]["out_tok"]        # [4, 128, 2048]
        for b in range(B):
            for hf in range(2):
                t0 = hf * 1024 + c * 128
                out[b, t0:t0 + 128, :] = ot[b * 2 + hf]
    return out


# revision 5
# speedup vs baseline: 1.3472x; 1.0398x over previous
"""Trainium2 Bass kernel for a dense transformer block (QKV+gate proj, RoPE,
QK-RMSNorm, causal SDPA, output-RMSNorm + SiLU gate, output projection).

Sharding: tensor-parallel over heads across 8 NeuronCores (2 heads/core) for
projections+attention; token-parallel output projection with replicated Wo.
The per-core attention outputs are exchanged with 4 small AllToAlls (one per
batch-half, ~0.5 MB/rank wire each) instead of AllGathers, and each core
computes the full 2048 output columns for its own 4x128-token chunks. Each
final projection is traced one attention-quarter after its AllToAll so the
strict PE FIFO never stalls on a collective.

Numerics: all matmuls bf16 (f32 accumulate). The output-RMSNorm eps term
(eps*denom^2) is dropped (validated <3e-3 output error), which removes all
softmax-denominator matmuls. rsqrt and silu are computed via exp/ln so the
whole kernel uses a single ACT table set (natural_log_exp_and_others).
"""

import os
import sys

for _p in ("/opt/trn_rl_repo", "/root/.axon_site/_ro/trn_rl_repo"):
    if os.path.isdir(_p) and _p not in sys.path:
        sys.path.insert(0, _p)

import numpy as np

import concourse.bass as bass
import concourse.mybir as mybir
from concourse import bacc
from concourse.bass_utils import run_bass_kernel_spmd
from concourse.tile import TileContext

B, T, HID = 2, 2048, 2048
H, D = 16, 128
NCORES = 8
HC = H // NCORES          # heads per core = 2
DC = HC * D               # 256 head-dims per core
BT = B * T                # 4096 tokens
KT = HID // 128           # 16 contraction tiles
SCALE = 1.0 / float(np.sqrt(D))
NEG = -3.0e38

F32 = mybir.dt.float32
BF16 = mybir.dt.bfloat16
AF = mybir.ActivationFunctionType
ALU = mybir.AluOpType

MMDT = BF16

LAST_EXEC_TIME_NS = None
_CACHED_NC = None


class _Bacc(bacc.Bacc):
    """Bacc with an ACT-table-set preference: serve Exp and Ln from the
    combined natural_log_exp_and_others set so alternating Ln/Exp chains
    (rsqrt-via-exp/ln, silu-via-exp/ln) don't thrash table loads."""

    def insert_act_table_loads(self):
        import bass_rust as _bass_rust
        from concourse.hw_specs import get_activation_tables
        has_activation = any(
            isinstance(i, mybir.InstActivation)
            for b in self.main_func.blocks
            for i in b.instructions
        )
        if not has_activation:
            return
        AFT = mybir.ActivationFunctionType
        tables = []
        for name, fns in get_activation_tables(self.m.arch).items():
            if name != "natural_log_exp_and_others":
                fns = fns - {AFT.Exp, AFT.Ln}
            tables.append((name, fns))
        _bass_rust.insert_act_table_loads(self, tables)


def _build_nc():
    nc = _Bacc("TRN2", target_bir_lowering=False, debug=False,
               num_devices=NCORES)

    xT = nc.dram_tensor("xT", [KT, BT // 1024, 128, 1024], MMDT,
                        kind="ExternalInput").ap()
    wq = nc.dram_tensor("wq", [128, KT, DC], MMDT, kind="ExternalInput").ap()
    wk = nc.dram_tensor("wk", [128, KT, DC], MMDT, kind="ExternalInput").ap()
    wv = nc.dram_tensor("wv", [128, KT, DC], MMDT, kind="ExternalInput").ap()
    wg = nc.dram_tensor("wg", [128, KT, DC], MMDT, kind="ExternalInput").ap()
    # full (replicated) Wo with o_norm folded: [128, kd-tile, 2048 cols]
    wo = nc.dram_tensor("wo", [128, KT, HID], MMDT, kind="ExternalInput").ap()
    cos2 = nc.dram_tensor("cos2", [128, T], F32, kind="ExternalInput").ap()
    sin2 = nc.dram_tensor("sin2", [128, T], F32, kind="ExternalInput").ap()
    negm = nc.dram_tensor("negm", [128, 128], F32, kind="ExternalInput").ap()
    ones_in = nc.dram_tensor("ones_in", [128, 128], MMDT,
                             kind="ExternalInput").ap()
    qrw = nc.dram_tensor("qrw", [128, 1], F32, kind="ExternalInput").ap()
    krw = nc.dram_tensor("krw", [128, 1], F32, kind="ExternalInput").ap()

    # output: per (b, half) a [128 tok, 2048] f32 chunk
    out_tok = nc.dram_tensor("out_tok", [B * 2, 128, HID], F32,
                             kind="ExternalOutput").ap()

    # AllToAll buffers: per (b, half): [8 shards(128 tok), 256 hd, 128 tok]
    a2a_in = [nc.dram_tensor(f"a2a_in{i}", [NCORES, DC, 128], MMDT).ap()
              for i in range(B * 2)]
    a2a_out = [nc.dram_tensor(f"a2a_out{i}", [NCORES, DC, 128], MMDT).ap()
               for i in range(B * 2)]

    with TileContext(nc) as tc:
        with tc.tile_pool(name="const", bufs=1) as const:
            ones_r = const.tile([128, 128], MMDT)
            nc.scalar.dma_start(out=ones_r, in_=ones_in)
            negm_sb = const.tile([128, 128], F32)
            nc.scalar.dma_start(out=negm_sb, in_=negm)
            onesb = const.tile([128, 1], F32)
            nc.vector.memset(onesb, 1.0)

            with tc.tile_pool(name="persist", bufs=1) as persist, \
                 tc.tile_pool(name="weights", bufs=1) as wpool:
                # all projection weights loaded once, used by both sweeps
                wsb = {}
                for wn, w_ap in (("q", wq), ("k", wk), ("g", wg), ("v", wv)):
                    w_t = wpool.tile([128, KT, DC], MMDT, tag=f"w{wn}",
                                     name=f"w{wn}")
                    nc.sync.dma_start(out=w_t, in_=w_ap)
                    wsb[wn] = w_t

                # final (rope+rms applied) qT/kT per head [d, tok] bf16;
                # written raw by sweep A, rewritten in place by post-A
                qTf = [persist.tile([128, BT], MMDT, tag=f"qTf{m}",
                                    name=f"qTf{m}") for m in range(HC)]
                kTf = [persist.tile([128, BT], MMDT, tag=f"kTf{m}",
                                    name=f"kTf{m}") for m in range(HC)]
                # v in [tok, d] layout per head (filled by DMA transpose)
                v_sb = [persist.tile([128, BT // 128, 128], MMDT,
                                     tag=f"v{m}", name=f"v{m}")
                        for m in range(HC)]
                # silu(gate) per head [d, tok]
                sg_sb = [persist.tile([128, BT], MMDT, tag=f"sg{m}",
                                      name=f"sg{m}") for m in range(HC)]

                # ============ sweep A: q, k + fused post (rope+rms) ========
                with tc.tile_pool(name="swA_x", bufs=5) as xpool, \
                     tc.tile_pool(name="swA_ps", bufs=1, space="PSUM") as pps, \
                     tc.tile_pool(name="swA_ss", bufs=2, space="PSUM") as pss, \
                     tc.tile_pool(name="swA_t", bufs=3) as tpool, \
                     tc.tile_pool(name="ropec", bufs=1) as rp:
                    cos_sb = rp.tile([128, T], F32)
                    nc.sync.dma_start(out=cos_sb, in_=cos2)
                    sin_sb = rp.tile([128, T], F32)
                    nc.sync.dma_start(out=sin_sb, in_=sin2)
                    qrw_sb = rp.tile([128, 1], F32)
                    nc.sync.dma_start(out=qrw_sb, in_=qrw)
                    krw_sb = rp.tile([128, 1], F32)
                    nc.sync.dma_start(out=krw_sb, in_=krw)
                    wsc = [qrw_sb, krw_sb]

                    for nbb in range(BT // 1024):
                        xch = []
                        for c in range(4):
                            xc = xpool.tile([128, 4, 1024], MMDT, tag="xc",
                                            name="xc")
                            nc.sync.dma_start(
                                out=xc,
                                in_=xT[4 * c:4 * c + 4, nbb, :, :].rearrange(
                                    "k p t -> p k t"))
                            xch.append(xc)
                        for half in range(2):
                            nb = 2 * nbb + half
                            c0 = nb * 512
                            ct0 = c0 % T       # within-batch token base
                            hs = slice(half * 512, (half + 1) * 512)
                            ps = {}
                            for mi in range(2):
                                for m in range(HC):
                                    # q tags double-buffered so the next
                                    # block's first MMs never wait on evac
                                    ps[(mi, m)] = pps.tile(
                                        [128, 512], F32, tag=f"pp{mi}{m}",
                                        name=f"pp{mi}{m}",
                                        bufs=(2 if mi == 0 else 1))
                            for k in range(KT):
                                xk = xch[k // 4][:, k % 4, hs]
                                for mi, wn in enumerate(("q", "k")):
                                    for m in range(HC):
                                        nc.tensor.matmul(
                                            ps[(mi, m)],
                                            wsb[wn][:, k,
                                                    m * 128:(m + 1) * 128],
                                            xk, start=(k == 0),
                                            stop=(k == KT - 1))
                            # evacuate raw bf16 (same order as next block MMs)
                            dests = [qTf, kTf]
                            mms = [(mi, m) for mi in range(2)
                                   for m in range(HC)]
                            for mi, m in mms:
                                nc.vector.tensor_copy(
                                    dests[mi][m][:, c0:c0 + 512],
                                    ps[(mi, m)])
                            # ---- post: rms factor + rope, in place.
                            # Ops with long dependency chains (stt) go at
                            # the END of the DVE stream so they never
                            # block the next block's evacuations.
                            raws = {k2: dests[k2[0]][k2[1]][:, c0:c0 + 512]
                                    for k2 in mms}
                            facs, ros = {}, {}
                            for mi, m in mms:
                                raw = raws[(mi, m)]
                                sq = tpool.tile([128, 512], MMDT,
                                                tag="sq", name="sq", bufs=4)
                                nc.vector.tensor_mul(sq, raw, raw)
                                ss = pss.tile([128, 512], F32, tag="ss",
                                              name="ss")
                                nc.tensor.matmul(ss, ones_r, sq,
                                                 start=True, stop=True)
                                # fac = rsqrt(ss/D) = exp(-0.5*ln(ss/D))
                                lnt = tpool.tile([128, 512], F32,
                                                 tag="lnt", name="lnt",
                                                 bufs=4)
                                nc.scalar.activation(
                                    out=lnt, in_=ss, func=AF.Ln,
                                    scale=1.0 / float(D))
                                fac = tpool.tile([128, 512], F32,
                                                 tag="fac", name="fac",
                                                 bufs=4)
                                nc.scalar.activation(
                                    out=fac, in_=lnt, func=AF.Exp,
                                    scale=-0.5)
                                facs[(mi, m)] = fac
                            cc = cos_sb[:, ct0:ct0 + 512]
                            ssn = sin_sb[:, ct0:ct0 + 512]
                            for mi, m in mms:
                                raw = raws[(mi, m)]
                                # rope: swap halves via SBUF DMA; sin_sb
                                # top half is pre-negated so ro =
                                # raw*cos + swap(raw)*sin' is full-width
                                sw = tpool.tile([128, 512], MMDT,
                                                tag="sw", name="sw", bufs=4)
                                nc.scalar.dma_start(out=sw[0:64, :],
                                                    in_=raw[64:128, :])
                                nc.scalar.dma_start(out=sw[64:128, :],
                                                    in_=raw[0:64, :])
                                u = tpool.tile([128, 512], F32, tag="u",
                                               name="u", bufs=4)
                                w2 = tpool.tile([128, 512], F32,
                                                tag="w2", name="w2", bufs=4)
                                nc.vector.tensor_mul(u, raw, cc)
                                nc.gpsimd.tensor_mul(w2, sw, ssn)
                                ro = tpool.tile([128, 512], F32,
                                                tag="ro", name="ro", bufs=4)
                                nc.gpsimd.tensor_add(ro, u, w2)
                                ros[(mi, m)] = ro
                            for mi, m in mms:
                                # dest = (ro * w[d]) * fac (fused)
                                nc.vector.scalar_tensor_tensor(
                                    out=raws[(mi, m)], in0=ros[(mi, m)],
                                    scalar=wsc[mi], in1=facs[(mi, m)],
                                    op0=ALU.mult, op1=ALU.mult)

                # ============ sweep B: gate, v ============
                with tc.tile_pool(name="swB_x", bufs=5) as xpool, \
                     tc.tile_pool(name="swB_ps", bufs=2, space="PSUM") as pps, \
                     tc.tile_pool(name="swB_t", bufs=3) as tpool, \
                     tc.tile_pool(name="vflat", bufs=1) as vfp:
                    vflat = [vfp.tile([128, BT], MMDT, tag=f"vf{m}",
                                      name=f"vf{m}") for m in range(HC)]

                    for nbb in range(BT // 1024):
                        xch = []
                        for c in range(4):
                            xc = xpool.tile([128, 4, 1024], MMDT, tag="xc",
                                            name="xc")
                            nc.sync.dma_start(
                                out=xc,
                                in_=xT[4 * c:4 * c + 4, nbb, :, :].rearrange(
                                    "k p t -> p k t"))
                            xch.append(xc)
                        # pair-interleaved: each weight load feeds both
                        # 512-token halves back to back (8 psum banks)
                        ps = {}
                        for mi in range(2):
                            for m in range(HC):
                                for half in range(2):
                                    ps[(mi, m, half)] = pps.tile(
                                        [128, 512], F32,
                                        tag=f"pp{mi}{m}{half}",
                                        name=f"pp{mi}{m}{half}", bufs=1)
                        for k in range(KT):
                            for mi, wn in enumerate(("g", "v")):
                                for m in range(HC):
                                    for half in range(2):
                                        hs = slice(half * 512,
                                                   (half + 1) * 512)
                                        nc.tensor.matmul(
                                            ps[(mi, m, half)],
                                            wsb[wn][:, k,
                                                    m * 128:(m + 1) * 128],
                                            xch[k // 4][:, k % 4, hs],
                                            start=(k == 0),
                                            stop=(k == KT - 1))
                        # all evacuations first (g m0 h0 leads: it is the
                        # next pair's first MM target), silu chains after
                        # so their ACT-dependent muls never block evacs
                        graws = {}
                        for half in range(2):
                            c0 = (2 * nbb + half) * 512
                            for m in range(HC):
                                graw = tpool.tile([128, 512], F32,
                                                  tag="graw", name="graw",
                                                  bufs=6)
                                nc.vector.tensor_copy(graw, ps[(0, m, half)])
                                graws[(half, m)] = graw
                                nc.vector.tensor_copy(
                                    vflat[m][:, c0:c0 + 512],
                                    ps[(1, m, half)])
                        for half in range(2):
                            nb = 2 * nbb + half
                            c0 = nb * 512
                            for m in range(HC):
                                # gate: silu via exp/ln (single table set)
                                graw = graws[(half, m)]
                                e1 = tpool.tile([128, 512], F32, tag="e1",
                                                name="e1")
                                nc.scalar.activation(out=e1, in_=graw,
                                                     func=AF.Exp, scale=-1.0)
                                l1 = tpool.tile([128, 512], F32, tag="l1",
                                                name="l1")
                                nc.scalar.activation(out=l1, in_=e1,
                                                     func=AF.Ln,
                                                     scale=1.0, bias=onesb)
                                s1 = tpool.tile([128, 512], F32, tag="s1",
                                                name="s1")
                                nc.scalar.activation(out=s1, in_=l1,
                                                     func=AF.Exp, scale=-1.0)
                                nc.vector.tensor_mul(
                                    sg_sb[m][:, c0:c0 + 512], graw, s1)
                            for m in range(HC):
                                for j in range(4):
                                    jj = nb * 4 + j
                                    nc.sync.dma_start(
                                        out=v_sb[m][:, jj, :],
                                        in_=vflat[m][:,
                                                     jj * 128:(jj + 1) * 128],
                                        transpose=True)

                # ============ attention + A2A + token-parallel out proj ====
                with tc.tile_pool(name="at_st", bufs=2, space="PSUM") as pst, \
                     tc.tile_pool(name="at_yt", bufs=2, space="PSUM") as pyt, \
                     tc.tile_pool(name="fin_ps", bufs=1, space="PSUM") as pfo, \
                     tc.tile_pool(name="at_es", bufs=4) as espool, \
                     tc.tile_pool(name="at_t", bufs=3) as tpool, \
                     tc.tile_pool(name="fin_w", bufs=1) as fwp, \
                     tc.tile_pool(name="fin_y", bufs=2) as fyp, \
                     tc.tile_pool(name="fin_o", bufs=2) as fop:
                    wo_sb = fwp.tile([128, KT, HID], MMDT, tag="wo")
                    nc.sync.dma_start(out=wo_sb, in_=wo)

                    def attention_quarter(b, hf):
                        t0 = b * T
                        ai = b * 2 + hf
                        for qb in (2 * hf, 2 * hf + 1):
                            for m in range(HC):
                                nk = 4 * (qb + 1)
                                q0 = t0 + qb * 512
                                ytp = pyt.tile([128, 512], F32, tag="yt",
                                               name="ytp")
                                for i2 in range(0, nk, 2):
                                    stp = pst.tile([128, 1024], F32,
                                                   tag="st", name="stp")
                                    e = espool.tile([128, 1024], MMDT,
                                                    tag="es", name="es")
                                    for j in range(2):
                                        i = i2 + j
                                        sl = slice(j * 512, (j + 1) * 512)
                                        nc.tensor.matmul(
                                            stp[:, sl],
                                            kTf[m][:, t0 + i * 128:
                                                   t0 + (i + 1) * 128],
                                            qTf[m][:, q0:q0 + 512],
                                            start=True, stop=True)
                                        q_off = i * 128 - qb * 512
                                        if q_off >= 0:
                                            nc.vector.tensor_add(
                                                stp[:, j * 512 + q_off:
                                                    j * 512 + q_off + 128],
                                                stp[:, j * 512 + q_off:
                                                    j * 512 + q_off + 128],
                                                negm_sb)
                                            if j == 0 and q_off > 0:
                                                nc.vector.memset(
                                                    e[:, 0:q_off], 0.0)
                                            if j == 1 and q_off > 0:
                                                nc.vector.memset(
                                                    stp[:, 512:512 + q_off],
                                                    NEG)
                                    q_off0 = i2 * 128 - qb * 512
                                    lo = max(0, q_off0)
                                    nc.scalar.activation(
                                        out=e[:, lo:], in_=stp[:, lo:],
                                        func=AF.Exp, scale=SCALE)
                                    for j in range(2):
                                        i = i2 + j
                                        sl = slice(j * 512, (j + 1) * 512)
                                        nc.tensor.matmul(
                                            ytp, v_sb[m][:, b * 16 + i, :],
                                            e[:, sl], start=(i == 0),
                                            stop=(i == nk - 1))
                                # tail: f = rsqrt(mean_d u^2), y = u*f*sg
                                ystash = tpool.tile([128, 512], MMDT,
                                                    tag="yst", name="ystash")
                                nc.vector.tensor_copy(ystash, ytp)
                                # square via psum x sbuf-stash (DVE allows
                                # only one PSUM operand per instruction)
                                sq = tpool.tile([128, 512], MMDT,
                                                tag="ysq", name="ysq")
                                nc.vector.tensor_mul(sq, ytp, ystash)
                                ssy = pyt.tile([128, 512], F32, tag="yt",
                                               name="ssy")
                                nc.tensor.matmul(ssy, ones_r, sq,
                                                 start=True, stop=True)
                                lny = tpool.tile([128, 512], F32,
                                                 tag="lny", name="lny")
                                nc.scalar.activation(out=lny, in_=ssy,
                                                     func=AF.Ln,
                                                     scale=1.0 / float(D))
                                fy = tpool.tile([128, 512], F32,
                                                tag="fy", name="fy")
                                nc.scalar.activation(out=fy, in_=lny,
                                                     func=AF.Exp, scale=-0.5)
                                yf1 = tpool.tile([128, 512], F32,
                                                 tag="yf1", name="yf1")
                                nc.vector.tensor_mul(yf1, ystash, fy)
                                yf = tpool.tile([128, 512], MMDT,
                                                tag="yf", name="yf")
                                nc.vector.tensor_mul(
                                    yf, yf1,
                                    sg_sb[m][:, t0 + qb * 512:
                                             t0 + (qb + 1) * 512])
                                # store into A2A input: 4 shards of 128
                                sh0 = 4 * (qb % 2)
                                nc.sync.dma_start(
                                    out=a2a_in[ai][sh0:sh0 + 4,
                                                   m * 128:(m + 1) * 128,
                                                   :].rearrange(
                                        "s p t -> p s t"),
                                    in_=yf)
                        nc.gpsimd.collective_compute(
                            "AllToAll", ALU.bypass,
                            ins=[a2a_in[ai]], outs=[a2a_out[ai]],
                            replica_groups=[list(range(NCORES))],
                        )

                    def final_chunk(ai):
                        # token-parallel out projection for chunk ai
                        yg = fyp.tile([128, KT, 128], MMDT, tag="yg",
                                      name="yg")
                        nc.sync.dma_start(
                            out=yg,
                            in_=a2a_out[ai].rearrange(
                                "r (h p) t -> p (r h) t", p=128))
                        for cp in range(2):   # column passes of 1024
                            fo = [pfo.tile([128, 512], F32,
                                           tag=f"fo{cc}", name=f"fo{cc}")
                                  for cc in range(2)]
                            for kd in range(KT):
                                for cc in range(2):
                                    col = cp * 1024 + cc * 512
                                    nc.tensor.matmul(
                                        fo[cc], yg[:, kd, :],
                                        wo_sb[:, kd, col:col + 512],
                                        start=(kd == 0),
                                        stop=(kd == KT - 1))
                            for cc in range(2):
                                ot = fop.tile([128, 512], F32, tag="ot",
                                              name="ot")
                                nc.vector.tensor_copy(ot, fo[cc])
                                nc.sync.dma_start(
                                    out=out_tok[ai, :,
                                                cp * 1024 + cc * 512:
                                                cp * 1024 + cc * 512 + 512],
                                    in_=ot)

                    # finals delayed one quarter behind their A2A so the
                    # strict PE FIFO never waits on an in-flight collective
                    quarters = [(b, hf) for b in range(B) for hf in range(2)]
                    for qi, (b, hf) in enumerate(quarters):
                        attention_quarter(b, hf)
                        if qi >= 1:
                            final_chunk(qi - 1)
                    final_chunk(3)
    nc.compile()
    return nc


def _get_nc():
    global _CACHED_NC
    if _CACHED_NC is None:
        _CACHED_NC = _build_nc()
    return _CACHED_NC


def kernel(x, Wq, Wk, Wv, Wg, Wo, q_rms_w, k_rms_w, o_norm_w):
    global LAST_EXEC_TIME_NS
    import ml_dtypes
    npdt = ml_dtypes.bfloat16
    x = np.asarray(x, dtype=np.float32)
    Wq = np.asarray(Wq, dtype=np.float32)
    Wk = np.asarray(Wk, dtype=np.float32)
    Wv = np.asarray(Wv, dtype=np.float32)
    Wg = np.asarray(Wg, dtype=np.float32)
    Wo = np.asarray(Wo, dtype=np.float32)
    q_rms_w = np.asarray(q_rms_w, dtype=np.float32)
    k_rms_w = np.asarray(k_rms_w, dtype=np.float32)
    o_norm_w = np.asarray(o_norm_w, dtype=np.float32)

    xT = x.reshape(BT, HID).T          # [HID, BT]
    xt4 = np.ascontiguousarray(
        xT.reshape(KT, 128, BT // 1024, 1024).transpose(0, 2, 1, 3)).astype(npdt)
    # fold o_norm_w into Wo rows: (y*o_w) @ Wo == y @ (o_w[:,None]*Wo)
    wo_scaled = Wo * np.tile(o_norm_w, H)[:, None]
    wo_t = np.ascontiguousarray(
        wo_scaled.reshape(KT, 128, HID).transpose(1, 0, 2)).astype(npdt)

    inv = 1.0 / (10000.0 ** (np.arange(0, D, 2, dtype=np.float64) / D))
    pos = np.arange(T, dtype=np.float64)
    fr = pos[:, None] * inv[None, :]          # [T, 64]
    cosT = np.cos(fr).T.astype(np.float32)    # [64, T]
    sinT = np.sin(fr).T.astype(np.float32)
    cos2 = np.ascontiguousarray(np.vstack([cosT, cosT]))   # [128, T]
    # top half negated: rope becomes raw*cos + swap(raw)*sin' on all
    # 128 partitions with a single full-width add
    sin2 = np.ascontiguousarray(np.vstack([-sinT, sinT]))

    kk, qq = np.meshgrid(np.arange(128), np.arange(128), indexing="ij")
    negm = np.where(kk <= qq, 0.0, NEG).astype(np.float32)
    ones128 = np.ones((128, 128), dtype=np.float32)

    in_maps = []
    for c in range(NCORES):
        csl = slice(c * DC, (c + 1) * DC)

        def wt(wmat):
            # [HID, DC] -> [128, KT, DC] matching the SBUF tile layout
            return np.ascontiguousarray(
                wmat[:, csl].reshape(KT, 128, DC).transpose(1, 0, 2)).astype(npdt)
        in_maps.append({
            "xT": xt4,
            "wq": wt(Wq),
            "wk": wt(Wk),
            "wv": wt(Wv),
            "wg": wt(Wg),
            "wo": wo_t,
            "cos2": cos2,
            "sin2": sin2,
            "negm": negm,
            "ones_in": ones128.astype(npdt),
            "qrw": np.ascontiguousarray(q_rms_w.reshape(128, 1)),
            "krw": np.ascontiguousarray(k_rms_w.reshape(128, 1)),
        })

    nc = _get_nc()
    trace = os.environ.get("KERNEL_TRACE", "0") == "1"
    res = run_bass_kernel_spmd(nc, in_maps, list(range(NCORES)), trace=trace)
    LAST_EXEC_TIME_NS = res.exec_time_ns

    out = np.empty((B, T, HID), dtype=np.float32)
    for c in range(NCORES):
        ot = res.results[c]["out_tok"]        # [4, 128, 2048]
        for b in range(B):
            for hf in range(2):
                t0 = hf * 1024 + c * 128
                out[b, t0:t0 + 128, :] = ot[b * 2 + hf]
    return out
